# revision 15
# baseline (speedup 1.0000x reference)
"""TRN2 Bass kernel: causal self-attention with adaptive row weights.

Reference computation (per batch element b):
    qkv = x @ Wv + bv                      # q = k = qkv (shared weights)
    v   = qkv * rw[:, None]
    att[i,j] = (qkv[i] . qkv[j]) * rw[j] / sqrt(hd)   per head
    att masked causal (j <= i), softmax over j
    y   = att @ v  (per head), concat heads
    out = y @ Wproj + bproj

Sharding: pure data parallel, one batch element per NeuronCore (B == 8).

Per-core dataflow (T=1024, C=1024, H=16, hd=64, P=128):
    x   --PE transpose-->  xT   [c', t]            (f32r)
    qkvT[c,t] = Wv.T @ xT + bv                     (f32r matmuls, psum->SBUF)
    v2[t, h*65:(h+1)*65] = [qkv*rw | ones]         (PE transpose of qkvT +
                                                    DVE per-partition rw scale,
                                                    bf16; ones col for denom)
    per head-pair g (heads 2g, 2g+1 live in qkvT partition rows of tile g):
      attT[j,i] = qkvT_h[:, j-blk].T @ qkvT_h[:, i-span]   (K=64 f32r)
      pT = exp(rw[j] * 0.125 * attT)  (ACT per-partition scale, bf16 out)
      diag blocks *= upper-triangular mask (keep j <= i)
      per i-tile: y_num[i, d], denom[i] = pT.T @ v2   (bf16, ones col -> denom)
      y = y_num * recip(denom)  (per-partition)  --PE transpose--> yT
    out = yT.T @ Wproj + bproj                     (f32r matmuls)
"""

import os
import numpy as np
import ml_dtypes

import concourse.bass as bass
import concourse.mybir as mybir
import concourse.tile as tile
from concourse import bacc
from concourse.bass_utils import run_bass_kernel_spmd

dt = mybir.dt
AF = mybir.ActivationFunctionType

B, T, C, H, HD = 8, 1024, 1024, 16, 64
P = 128
NT = T // P          # 8 t-tiles
NC_ = C // P         # 8 c-tiles
NPAIR = H // 2       # 8 head pairs
SCALE = 1.0 / np.sqrt(HD)  # 0.125

# bank-aligned i-chunks (offset, width) per j-tile; psum bank = 512 f32
def _chunks(jt):
    if jt <= 3:
        return [(jt * P, 512 - jt * P), (512, 512)]
    return [(jt * P, 1024 - jt * P)]


def build_nc():
    nc = bacc.Bacc("TRN2", target_bir_lowering=False, debug=False,
                   num_devices=B)
    x = nc.dram_tensor("x", [T, C], dt.float32, kind="ExternalInput").ap()
    rw_col_in = nc.dram_tensor("rw_col", [P, T // P], dt.float32,
                               kind="ExternalInput").ap()
    rwq_in = nc.dram_tensor("rwq", [P, T // P], dt.float32,
                            kind="ExternalInput").ap()
    wv = nc.dram_tensor("wv", [C, C], dt.float32, kind="ExternalInput").ap()
    bv = nc.dram_tensor("bv", [C], dt.float32, kind="ExternalInput").ap()
    wp = nc.dram_tensor("wp", [C, C], dt.float32, kind="ExternalInput").ap()
    bp = nc.dram_tensor("bp", [C], dt.float32, kind="ExternalInput").ap()
    ident = nc.dram_tensor("ident", [P, P], dt.float32,
                           kind="ExternalInput").ap()
    mask2 = nc.dram_tensor("mask2", [P, 2 * P], dt.bfloat16,
                           kind="ExternalInput").ap()
    out = nc.dram_tensor("out", [T, C], dt.float32, kind="ExternalOutput").ap()

    f32r = dt.float32r
    with tile.TileContext(nc) as tc:
        with (
            tc.tile_pool(name="const", bufs=1) as cpool,
            tc.tile_pool(name="qkvT", bufs=1) as qkvp,
            tc.tile_pool(name="v2", bufs=1) as v2p,
            tc.tile_pool(name="pT", bufs=9) as pTp,
            tc.tile_pool(name="yT", bufs=1) as yTp,
            tc.tile_pool(name="small", bufs=2) as smp,
            tc.tile_pool(name="fout", bufs=2) as fop,
            tc.tile_pool(name="ps_a", bufs=2, space="PSUM") as psa,
            tc.tile_pool(name="ps_att", bufs=2, space="PSUM") as psatt,
            tc.tile_pool(name="ps_av", bufs=2, space="PSUM") as psav,
        ):
            # ---- constants ----
            idr = cpool.tile([P, P], f32r)
            nc.sync.dma_start(idr[:], ident.bitcast(f32r))
            msk = cpool.tile([P, 2 * P], dt.bfloat16)
            nc.sync.dma_start(msk[:], mask2)
            mskv = msk[:].rearrange("p (h c) -> p h c", c=P)
            rw_col = cpool.tile([P, NT], dt.float32)
            nc.sync.dma_start(rw_col[:], rw_col_in)
            rwq = cpool.tile([P, NT], dt.float32)
            nc.sync.dma_start(rwq[:], rwq_in)

            qkvT = qkvp.tile([P, NC_, T], f32r)       # [c-part, ct, t]
            v2 = v2p.tile([P, NT, H * (HD + 1)], dt.bfloat16)
            yT = yTp.tile([P, NC_, T], f32r)          # [c-part, ct, t]

            # ---- phase 1: load x/Wv, transpose x, qkv matmuls ----
            with (
                tc.tile_pool(name="wv", bufs=1) as wvp,
                tc.tile_pool(name="xT", bufs=1) as xTp,
                tc.tile_pool(name="xr", bufs=2) as xrp,
            ):
                wvt = wvp.tile([P, NC_, C], f32r)     # [c'-part, k, c]
                for k in range(NC_):
                    nc.scalar.dma_start(wvt[:, k, :],
                                        wv[k * P:(k + 1) * P, :].bitcast(f32r))
                xT = xTp.tile([P, NC_, T], f32r)      # [c'-part, k, t]
                for tt in range(NT):
                    xr = xrp.tile([P, C], f32r)
                    nc.gpsimd.dma_start(xr[:],
                                      x[tt * P:(tt + 1) * P, :].bitcast(f32r))
                    for ct in range(NC_):
                        pt = psa.tile([P, P], f32r, tag="misc")
                        nc.tensor.transpose(pt[:], xr[:, ct * P:(ct + 1) * P],
                                            idr[:])
                        nc.any.tensor_copy(xT[:, ct, tt * P:(tt + 1) * P],
                                           pt[:].bitcast(dt.float32))

                # qkvT[c, t] = Wv.T @ xT + bv  (per ct, two 512 chunks)
                for ct in range(NC_):
                    for ch in range(2):
                        sl = slice(ch * 512, (ch + 1) * 512)
                        pq = psa.tile([P, 512], dt.float32, tag="misc")
                        for k in range(NC_):
                            nc.tensor.matmul(
                                pq[:], wvt[:, k, ct * P:(ct + 1) * P],
                                xT[:, k, sl], start=(k == 0),
                                stop=(k == NC_ - 1))
                        nc.any.tensor_copy(qkvT[:, ct, sl], pq[:])

                    # v2 blocks for this ct: transpose qkvT[ct] tiles
                    for tt in range(NT):
                        pt2 = psa.tile([P, P], f32r, tag="misc")
                        nc.tensor.transpose(
                            pt2[:], qkvT[:, ct, tt * P:(tt + 1) * P], idr[:])
                        dst = (v2[:, tt, :]
                               .rearrange("p (h c) -> p h c", c=HD + 1)
                               [:, 2 * ct:2 * ct + 2, 0:HD])
                        src = pt2[:].bitcast(dt.float32).rearrange(
                            "p (h c) -> p h c", c=HD)
                        nc.vector.tensor_scalar_mul(dst, src,
                                                    rw_col[:, tt:tt + 1])

            # ones columns of v2 (denominator trick)
            for tt in range(NT):
                v2h = v2[:, tt, :].rearrange("p (h c) -> p h c", c=HD + 1)
                nc.gpsimd.memset(v2h[:, :, HD:HD + 1], 1.0)

            # ---- phase 2: attention per head pair ----
            with tc.tile_pool(name="wp", bufs=1) as wpp:
                wpt = wpp.tile([P, NC_, C], f32r)     # [ci-part, k, co]
                for k in range(NC_):
                    nc.sync.dma_start(wpt[:, k, :],
                                      wp[k * P:(k + 1) * P, :].bitcast(f32r))

                for g in range(NPAIR):
                    pT_tiles = []
                    for jt in range(NT):
                        pt_t = pTp.tile([P, 2, T], dt.bfloat16, tag="pT")
                        pT_tiles.append(pt_t)
                        for (off, cw) in _chunks(jt):
                            pa = psatt.tile([P, 2, 512], dt.float32,
                                            tag="att")
                            for hh in range(2):
                                lo = hh * HD
                                nc.tensor.matmul(
                                    pa[:, hh, 0:cw],
                                    qkvT[lo:lo + HD, g,
                                         jt * P:(jt + 1) * P],
                                    qkvT[lo:lo + HD, g, off:off + cw],
                                    start=True, stop=True)
                            nc.scalar.activation(
                                pt_t[:, :, off:off + cw], pa[:, :, 0:cw],
                                AF.Exp, bias=0.0, scale=rwq[:, jt:jt + 1])
                        # mask the diagonal block (keep j <= i)
                        nc.vector.tensor_mul(
                            pt_t[:, :, jt * P:(jt + 1) * P],
                            pt_t[:, :, jt * P:(jt + 1) * P], mskv)

                    # av in [65, i] orientation: few large N=512 matmuls.
                    # Row 64 of psum = softmax denominator (ones column).
                    # Varying-slice accumulation skips unwritten pT regions.
                    for hh in range(2):
                        h = 2 * g + hh
                        for c in range(2):
                            py = psav.tile([P, 512], dt.float32, tag="av")
                            last = min(4 * c + 3, NT - 1)
                            for jt in range(last + 1):
                                a = max(c * 512, jt * P)
                                nc.tensor.matmul(
                                    py[0:HD + 1, a - c * 512:512],
                                    v2[:, jt, h * (HD + 1):
                                       (h + 1) * (HD + 1)],
                                    pT_tiles[jt][:, hh, a:(c + 1) * 512],
                                    start=(jt == 0), stop=(jt == last))
                            drow = smp.tile([1, 512], dt.float32, tag="dr")
                            nc.vector.tensor_copy(drow[:], py[HD:HD + 1, :])
                            rrow = smp.tile([1, 512], dt.float32, tag="rr")
                            nc.vector.reciprocal_approx_fast(
                                rrow[:], drow[:])
                            bc = smp.tile([HD, 512], dt.float32, tag="bc")
                            nc.gpsimd.partition_broadcast(bc[:], rrow[:])
                            nc.vector.tensor_mul(
                                yT[hh * HD:(hh + 1) * HD, g,
                                   c * 512:(c + 1) * 512],
                                py[0:HD, :], bc[:])

                # ---- phase 3: proj ----
                for tt in range(NT):
                    for ch in range(2):
                        sl = slice(ch * 512, (ch + 1) * 512)
                        po = psa.tile([P, 512], dt.float32, tag="misc")
                        for k in range(NC_):
                            nc.tensor.matmul(
                                po[:], yT[:, k, tt * P:(tt + 1) * P],
                                wpt[:, k, sl], start=(k == 0),
                                stop=(k == NC_ - 1))
                        fo = fop.tile([P, 512], dt.float32, tag="fo")
                        nc.any.tensor_copy(fo[:], po[:])
                        nc.sync.dma_start(out[tt * P:(tt + 1) * P, sl], fo[:])

    nc.compile()
    return nc


_NC_CACHE = None


def _get_nc():
    global _NC_CACHE
    if _NC_CACHE is None:
        _NC_CACHE = build_nc()
    return _NC_CACHE


def _consts():
    tri = np.triu(np.ones((P, P), np.float32))  # keep j <= i
    return {
        "ident": np.eye(P, dtype=np.float32),
        "mask2": np.concatenate([tri, tri], axis=1)
                   .astype(ml_dtypes.bfloat16),
    }


def run(inputs, trace=False, tmpdir=None):
    x = np.asarray(inputs["x"], dtype=np.float32)
    rw = np.asarray(inputs["row_weights"], dtype=np.float32)
    wv = np.asarray(inputs["Wv"], dtype=np.float32)
    bv = np.asarray(inputs["bv"], dtype=np.float32)
    wp = np.asarray(inputs["Wproj"], dtype=np.float32)
    bp = np.asarray(inputs["bproj"], dtype=np.float32)
    assert not bv.any() and not bp.any(), \
        "kernel specialized for zero biases (spec: fill=zeros)"
    consts = _consts()
    in_maps = []
    for b in range(B):
        rwc = np.ascontiguousarray(rw[b].reshape(8, 128).T)
        m = {"x": np.ascontiguousarray(x[b]),
             "rw_col": rwc, "rwq": rwc * np.float32(SCALE),
             "wv": wv, "bv": bv, "wp": wp, "bp": bp}
        m.update(consts)
        in_maps.append(m)
    nc = _get_nc()
    res = run_bass_kernel_spmd(nc, in_maps, list(range(B)), trace=trace,
                               tmpdir=tmpdir)
    out = np.stack([res.results[b]["out"] for b in range(B)])
    return out, res


def kernel(**inputs):
    out, _ = run(inputs, trace=False)
    return out


if __name__ == "__main__":
    rng = np.random.default_rng(0)
    ins = {
        "x": rng.standard_normal((B, T, C)).astype(np.float32),
        "row_weights": rng.uniform(0.5, 1.5, (B, T)).astype(np.float32),
        "Wv": (rng.standard_normal((C, C)) * 0.02).astype(np.float32),
        "bv": np.zeros(C, np.float32),
        "Wproj": (rng.standard_normal((C, C)) * 0.02).astype(np.float32),
        "bproj": np.zeros(C, np.float32),
    }
    out = kernel(**ins)
    print("kernel ran, out", out.shape, out.dtype)


# revision 17
# speedup vs baseline: 1.0494x; 1.0494x over previous
"""TRN2 Bass kernel: causal self-attention with adaptive row weights.

Reference computation (per batch element b):
    qkv = x @ Wv + bv                      # q = k = qkv (shared weights)
    v   = qkv * rw[:, None]
    att[i,j] = (qkv[i] . qkv[j]) * rw[j] / sqrt(hd)   per head
    att masked causal (j <= i), softmax over j
    y   = att @ v  (per head), concat heads
    out = y @ Wproj + bproj

Sharding: pure data parallel, one batch element per NeuronCore (B == 8).

Per-core dataflow (T=1024, C=1024, H=16, hd=64, P=128):
    x   --PE transpose-->  xT   [c', t]            (f32r)
    qkvT[c,t] = Wv.T @ xT + bv                     (f32r matmuls, psum->SBUF)
    v2[t, h*65:(h+1)*65] = [qkv*rw | ones]         (PE transpose of qkvT +
                                                    DVE per-partition rw scale,
                                                    bf16; ones col for denom)
    per head-pair g (heads 2g, 2g+1 live in qkvT partition rows of tile g):
      attT[j,i] = qkvT_h[:, j-blk].T @ qkvT_h[:, i-span]   (K=64 f32r)
      pT = exp(rw[j] * 0.125 * attT)  (ACT per-partition scale, bf16 out)
      diag blocks *= upper-triangular mask (keep j <= i)
      per i-tile: y_num[i, d], denom[i] = pT.T @ v2   (bf16, ones col -> denom)
      y = y_num * recip(denom)  (per-partition)  --PE transpose--> yT
    out = yT.T @ Wproj + bproj                     (f32r matmuls)
"""

import os
import numpy as np
import ml_dtypes

import concourse.bass as bass
import concourse.mybir as mybir
import concourse.tile as tile
from concourse import bacc
from concourse.bass_utils import run_bass_kernel_spmd

dt = mybir.dt
AF = mybir.ActivationFunctionType

B, T, C, H, HD = 8, 1024, 1024, 16, 64
P = 128
NT = T // P          # 8 t-tiles
NC_ = C // P         # 8 c-tiles
NPAIR = H // 2       # 8 head pairs
SCALE = 1.0 / np.sqrt(HD)  # 0.125

# bank-aligned i-chunks (offset, width) per j-tile; psum bank = 512 f32
def _chunks(jt):
    if jt <= 3:
        return [(jt * P, 512 - jt * P), (512, 512)]
    return [(jt * P, 1024 - jt * P)]


def build_nc():
    nc = bacc.Bacc("TRN2", target_bir_lowering=False, debug=False,
                   num_devices=B)
    x = nc.dram_tensor("x", [T, C], dt.float32, kind="ExternalInput").ap()
    rw_col_in = nc.dram_tensor("rw_col", [P, T // P], dt.float32,
                               kind="ExternalInput").ap()
    rwq_in = nc.dram_tensor("rwq", [P, T // P], dt.float32,
                            kind="ExternalInput").ap()
    wv = nc.dram_tensor("wv", [C, C], dt.float32, kind="ExternalInput").ap()
    bv = nc.dram_tensor("bv", [C], dt.float32, kind="ExternalInput").ap()
    wp = nc.dram_tensor("wp", [C, C], dt.float32, kind="ExternalInput").ap()
    bp = nc.dram_tensor("bp", [C], dt.float32, kind="ExternalInput").ap()
    ident = nc.dram_tensor("ident", [P, P], dt.float32,
                           kind="ExternalInput").ap()
    mask2 = nc.dram_tensor("mask2", [P, 2 * P], dt.bfloat16,
                           kind="ExternalInput").ap()
    out = nc.dram_tensor("out", [T, C], dt.float32, kind="ExternalOutput").ap()

    f32r = dt.float32r
    with tile.TileContext(nc) as tc:
        with (
            tc.tile_pool(name="const", bufs=1) as cpool,
            tc.tile_pool(name="qkvT", bufs=1) as qkvp,
            tc.tile_pool(name="v2", bufs=1) as v2p,
            tc.tile_pool(name="pT", bufs=9) as pTp,
            tc.tile_pool(name="yT", bufs=1) as yTp,
            tc.tile_pool(name="small", bufs=2) as smp,
            tc.tile_pool(name="fout", bufs=2) as fop,
            tc.tile_pool(name="ps_a", bufs=2, space="PSUM") as psa,
            tc.tile_pool(name="ps_att", bufs=2, space="PSUM") as psatt,
            tc.tile_pool(name="ps_av", bufs=2, space="PSUM") as psav,
        ):
            # ---- constants ----
            idr = cpool.tile([P, P], f32r)
            nc.sync.dma_start(idr[:], ident.bitcast(f32r))
            msk = cpool.tile([P, 2 * P], dt.bfloat16)
            nc.sync.dma_start(msk[:], mask2)
            mskv = msk[:].rearrange("p (h c) -> p h c", c=P)
            rw_col = cpool.tile([P, NT], dt.float32)
            nc.sync.dma_start(rw_col[:], rw_col_in)
            rwq = cpool.tile([P, NT], dt.float32)
            nc.sync.dma_start(rwq[:], rwq_in)

            qkvT = qkvp.tile([P, NC_, T], f32r)       # [c-part, ct, t]
            v2 = v2p.tile([P, NT, H * (HD + 1)], dt.bfloat16)
            yT = yTp.tile([P, NC_, T], f32r)          # [c-part, ct, t]

            # ---- phase 1: load x/Wv, transpose x, qkv matmuls ----
            with (
                tc.tile_pool(name="wv", bufs=1) as wvp,
                tc.tile_pool(name="xT", bufs=1) as xTp,
                tc.tile_pool(name="xr", bufs=2) as xrp,
            ):
                wvt = wvp.tile([P, NC_, C], f32r)     # [c'-part, k, c]
                for k in range(NC_):
                    nc.scalar.dma_start(wvt[:, k, :],
                                        wv[k * P:(k + 1) * P, :].bitcast(f32r))
                xT = xTp.tile([P, NC_, T], f32r)      # [c'-part, k, t]
                for tt in range(NT):
                    xr = xrp.tile([P, C], f32r)
                    nc.gpsimd.dma_start(xr[:],
                                      x[tt * P:(tt + 1) * P, :].bitcast(f32r))
                    for ct in range(NC_):
                        pt = psa.tile([P, P], f32r, tag="misc")
                        nc.tensor.transpose(pt[:], xr[:, ct * P:(ct + 1) * P],
                                            idr[:])
                        if ct % 2 == 0:
                            nc.vector.tensor_copy(
                                xT[:, ct, tt * P:(tt + 1) * P],
                                pt[:].bitcast(dt.float32))
                        else:
                            nc.scalar.copy(xT[:, ct, tt * P:(tt + 1) * P],
                                           pt[:].bitcast(dt.float32))

                # qkvT[c, t] = Wv.T @ xT + bv  (per ct, two 512 chunks)
                for ct in range(NC_):
                    for ch in range(2):
                        sl = slice(ch * 512, (ch + 1) * 512)
                        pq = psa.tile([P, 512], dt.float32, tag="misc")
                        for k in range(NC_):
                            nc.tensor.matmul(
                                pq[:], wvt[:, k, ct * P:(ct + 1) * P],
                                xT[:, k, sl], start=(k == 0),
                                stop=(k == NC_ - 1))
                        nc.vector.tensor_copy(qkvT[:, ct, sl], pq[:])

                    # v2 blocks for this ct: transpose qkvT[ct] tiles
                    for tt in range(NT):
                        pt2 = psa.tile([P, P], f32r, tag="misc")
                        nc.tensor.transpose(
                            pt2[:], qkvT[:, ct, tt * P:(tt + 1) * P], idr[:])
                        dst = (v2[:, tt, :]
                               .rearrange("p (h c) -> p h c", c=HD + 1)
                               [:, 2 * ct:2 * ct + 2, 0:HD])
                        src = pt2[:].bitcast(dt.float32).rearrange(
                            "p (h c) -> p h c", c=HD)
                        nc.vector.tensor_scalar_mul(dst, src,
                                                    rw_col[:, tt:tt + 1])

            # ones columns of v2 (denominator trick)
            for tt in range(NT):
                v2h = v2[:, tt, :].rearrange("p (h c) -> p h c", c=HD + 1)
                nc.gpsimd.memset(v2h[:, :, HD:HD + 1], 1.0)

            # ---- phase 2: attention per head pair ----
            with tc.tile_pool(name="wp", bufs=1) as wpp:
                wpt = wpp.tile([P, NC_, C], f32r)     # [ci-part, k, co]
                for k in range(NC_):
                    nc.sync.dma_start(wpt[:, k, :],
                                      wp[k * P:(k + 1) * P, :].bitcast(f32r))

                for g in range(NPAIR):
                    pT_tiles = []
                    for jt in range(NT):
                        pt_t = pTp.tile([P, 2, T], dt.bfloat16, tag="pT")
                        pT_tiles.append(pt_t)
                        for (off, cw) in _chunks(jt):
                            pa = psatt.tile([P, 2, 512], dt.float32,
                                            tag="att")
                            for hh in range(2):
                                lo = hh * HD
                                nc.tensor.matmul(
                                    pa[:, hh, 0:cw],
                                    qkvT[lo:lo + HD, g,
                                         jt * P:(jt + 1) * P],
                                    qkvT[lo:lo + HD, g, off:off + cw],
                                    start=True, stop=True)
                            nc.scalar.activation(
                                pt_t[:, :, off:off + cw], pa[:, :, 0:cw],
                                AF.Exp, bias=0.0, scale=rwq[:, jt:jt + 1])
                        # mask the diagonal block (keep j <= i)
                        nc.vector.tensor_mul(
                            pt_t[:, :, jt * P:(jt + 1) * P],
                            pt_t[:, :, jt * P:(jt + 1) * P], mskv)

                    # av in [65, i] orientation: few large N=512 matmuls.
                    # Row 64 of psum = softmax denominator (ones column).
                    # Varying-slice accumulation skips unwritten pT regions.
                    for hh in range(2):
                        h = 2 * g + hh
                        for c in range(2):
                            py = psav.tile([P, 512], dt.float32, tag="av")
                            last = min(4 * c + 3, NT - 1)
                            for jt in range(last + 1):
                                a = max(c * 512, jt * P)
                                nc.tensor.matmul(
                                    py[0:HD + 1, a - c * 512:512],
                                    v2[:, jt, h * (HD + 1):
                                       (h + 1) * (HD + 1)],
                                    pT_tiles[jt][:, hh, a:(c + 1) * 512],
                                    start=(jt == 0), stop=(jt == last))
                            drow = smp.tile([1, 512], dt.float32, tag="dr")
                            nc.vector.tensor_copy(drow[:], py[HD:HD + 1, :])
                            rrow = smp.tile([1, 512], dt.float32, tag="rr")
                            nc.vector.reciprocal_approx_fast(
                                rrow[:], drow[:])
                            bc = smp.tile([HD, 512], dt.float32, tag="bc")
                            nc.gpsimd.partition_broadcast(bc[:], rrow[:])
                            nc.vector.tensor_mul(
                                yT[hh * HD:(hh + 1) * HD, g,
                                   c * 512:(c + 1) * 512],
                                py[0:HD, :], bc[:])

                # ---- phase 3: proj ----
                for tt in range(NT):
                    for ch in range(2):
                        sl = slice(ch * 512, (ch + 1) * 512)
                        po = psa.tile([P, 512], dt.float32, tag="misc")
                        for k in range(NC_):
                            nc.tensor.matmul(
                                po[:], yT[:, k, tt * P:(tt + 1) * P],
                                wpt[:, k, sl], start=(k == 0),
                                stop=(k == NC_ - 1))
                        fo = fop.tile([P, 512], dt.float32, tag="fo")
                        nc.scalar.copy(fo[:], po[:])
                        nc.sync.dma_start(out[tt * P:(tt + 1) * P, sl], fo[:])

    nc.compile()
    return nc


_NC_CACHE = None


def _get_nc():
    global _NC_CACHE
    if _NC_CACHE is None:
        _NC_CACHE = build_nc()
    return _NC_CACHE


def _consts():
    tri = np.triu(np.ones((P, P), np.float32))  # keep j <= i
    return {
        "ident": np.eye(P, dtype=np.float32),
        "mask2": np.concatenate([tri, tri], axis=1)
                   .astype(ml_dtypes.bfloat16),
    }


def run(inputs, trace=False, tmpdir=None):
    x = np.asarray(inputs["x"], dtype=np.float32)
    rw = np.asarray(inputs["row_weights"], dtype=np.float32)
    wv = np.asarray(inputs["Wv"], dtype=np.float32)
    bv = np.asarray(inputs["bv"], dtype=np.float32)
    wp = np.asarray(inputs["Wproj"], dtype=np.float32)
    bp = np.asarray(inputs["bproj"], dtype=np.float32)
    assert not bv.any() and not bp.any(), \
        "kernel specialized for zero biases (spec: fill=zeros)"
    consts = _consts()
    in_maps = []
    for b in range(B):
        rwc = np.ascontiguousarray(rw[b].reshape(8, 128).T)
        m = {"x": np.ascontiguousarray(x[b]),
             "rw_col": rwc, "rwq": rwc * np.float32(SCALE),
             "wv": wv, "bv": bv, "wp": wp, "bp": bp}
        m.update(consts)
        in_maps.append(m)
    nc = _get_nc()
    res = run_bass_kernel_spmd(nc, in_maps, list(range(B)), trace=trace,
                               tmpdir=tmpdir)
    out = np.stack([res.results[b]["out"] for b in range(B)])
    return out, res


def kernel(**inputs):
    out, _ = run(inputs, trace=False)
    return out


if __name__ == "__main__":
    rng = np.random.default_rng(0)
    ins = {
        "x": rng.standard_normal((B, T, C)).astype(np.float32),
        "row_weights": rng.uniform(0.5, 1.5, (B, T)).astype(np.float32),
        "Wv": (rng.standard_normal((C, C)) * 0.02).astype(np.float32),
        "bv": np.zeros(C, np.float32),
        "Wproj": (rng.standard_normal((C, C)) * 0.02).astype(np.float32),
        "bproj": np.zeros(C, np.float32),
    }
    out = kernel(**ins)
    print("kernel ran, out", out.shape, out.dtype)


# revision 18
# speedup vs baseline: 1.0632x; 1.0131x over previous
"""TRN2 Bass kernel: causal self-attention with adaptive row weights.

Reference computation (per batch element b):
    qkv = x @ Wv + bv                      # q = k = qkv (shared weights)
    v   = qkv * rw[:, None]
    att[i,j] = (qkv[i] . qkv[j]) * rw[j] / sqrt(hd)   per head
    att masked causal (j <= i), softmax over j
    y   = att @ v  (per head), concat heads
    out = y @ Wproj + bproj

Sharding: pure data parallel, one batch element per NeuronCore (B == 8).

Per-core dataflow (T=1024, C=1024, H=16, hd=64, P=128):
    x   --PE transpose-->  xT   [c', t]            (f32r)
    qkvT[c,t] = Wv.T @ xT + bv                     (f32r matmuls, psum->SBUF)
    v2[t, h*65:(h+1)*65] = [qkv*rw | ones]         (PE transpose of qkvT +
                                                    DVE per-partition rw scale,
                                                    bf16; ones col for denom)
    per head-pair g (heads 2g, 2g+1 live in qkvT partition rows of tile g):
      attT[j,i] = qkvT_h[:, j-blk].T @ qkvT_h[:, i-span]   (K=64 f32r)
      pT = exp(rw[j] * 0.125 * attT)  (ACT per-partition scale, bf16 out)
      diag blocks *= upper-triangular mask (keep j <= i)
      per i-tile: y_num[i, d], denom[i] = pT.T @ v2   (bf16, ones col -> denom)
      y = y_num * recip(denom)  (per-partition)  --PE transpose--> yT
    out = yT.T @ Wproj + bproj                     (f32r matmuls)
"""

import os
import numpy as np
import ml_dtypes

import concourse.bass as bass
import concourse.mybir as mybir
import concourse.tile as tile
from concourse import bacc
from concourse.bass_utils import run_bass_kernel_spmd

dt = mybir.dt
AF = mybir.ActivationFunctionType

B, T, C, H, HD = 8, 1024, 1024, 16, 64
P = 128
NT = T // P          # 8 t-tiles
NC_ = C // P         # 8 c-tiles
NPAIR = H // 2       # 8 head pairs
SCALE = 1.0 / np.sqrt(HD)  # 0.125

# bank-aligned i-chunks (offset, width) per j-tile; psum bank = 512 f32
def _chunks(jt):
    if jt <= 3:
        return [(jt * P, 512 - jt * P), (512, 512)]
    return [(jt * P, 1024 - jt * P)]


def build_nc():
    nc = bacc.Bacc("TRN2", target_bir_lowering=False, debug=False,
                   num_devices=B)
    x = nc.dram_tensor("x", [T, C], dt.float32, kind="ExternalInput").ap()
    rw_col_in = nc.dram_tensor("rw_col", [P, T // P], dt.float32,
                               kind="ExternalInput").ap()
    rwq_in = nc.dram_tensor("rwq", [P, T // P], dt.float32,
                            kind="ExternalInput").ap()
    wv = nc.dram_tensor("wv", [C, C], dt.float32, kind="ExternalInput").ap()
    bv = nc.dram_tensor("bv", [C], dt.float32, kind="ExternalInput").ap()
    wp = nc.dram_tensor("wp", [C, C], dt.float32, kind="ExternalInput").ap()
    bp = nc.dram_tensor("bp", [C], dt.float32, kind="ExternalInput").ap()
    ident = nc.dram_tensor("ident", [P, P], dt.float32,
                           kind="ExternalInput").ap()
    mask2 = nc.dram_tensor("mask2", [P, 2 * P], dt.bfloat16,
                           kind="ExternalInput").ap()
    out = nc.dram_tensor("out", [T, C], dt.float32, kind="ExternalOutput").ap()

    f32r = dt.float32r
    with tile.TileContext(nc) as tc:
        with (
            tc.tile_pool(name="const", bufs=1) as cpool,
            tc.tile_pool(name="qkvT", bufs=1) as qkvp,
            tc.tile_pool(name="v2", bufs=1) as v2p,
            tc.tile_pool(name="pT", bufs=9) as pTp,
            tc.tile_pool(name="yT", bufs=1) as yTp,
            tc.tile_pool(name="small", bufs=2) as smp,
            tc.tile_pool(name="fout", bufs=2) as fop,
            tc.tile_pool(name="ps_a", bufs=2, space="PSUM") as psa,
            tc.tile_pool(name="ps_att", bufs=2, space="PSUM") as psatt,
            tc.tile_pool(name="ps_av", bufs=2, space="PSUM") as psav,
        ):
            # ---- constants ----
            idr = cpool.tile([P, P], f32r)
            nc.sync.dma_start(idr[:], ident.bitcast(f32r))
            msk = cpool.tile([P, 2 * P], dt.bfloat16)
            nc.sync.dma_start(msk[:], mask2)
            mskv = msk[:].rearrange("p (h c) -> p h c", c=P)
            rw_col = cpool.tile([P, NT], dt.float32)
            nc.sync.dma_start(rw_col[:], rw_col_in)
            rwq = cpool.tile([P, NT], dt.float32)
            nc.sync.dma_start(rwq[:], rwq_in)

            qkvT = qkvp.tile([P, NC_, T], f32r)       # [c-part, ct, t]
            v2 = v2p.tile([P, NT, H * (HD + 1)], dt.bfloat16)
            yT = yTp.tile([P, NC_, T], f32r)          # [c-part, ct, t]

            # ---- phase 1: load x/Wv, transpose x, qkv matmuls ----
            with (
                tc.tile_pool(name="wv", bufs=1) as wvp,
                tc.tile_pool(name="xT", bufs=1) as xTp,
                tc.tile_pool(name="xr", bufs=2) as xrp,
            ):
                wvt = wvp.tile([P, NC_, C], f32r)     # [c'-part, k, c]
                for k in range(NC_):
                    nc.scalar.dma_start(wvt[:, k, :],
                                        wv[k * P:(k + 1) * P, :].bitcast(f32r))
                xT = xTp.tile([P, NC_, T], f32r)      # [c'-part, k, t]
                for tt in range(NT):
                    xr = xrp.tile([P, C], f32r)
                    nc.gpsimd.dma_start(xr[:],
                                      x[tt * P:(tt + 1) * P, :].bitcast(f32r))
                    for ct in range(NC_):
                        pt = psa.tile([P, P], f32r, tag="misc")
                        nc.tensor.transpose(pt[:], xr[:, ct * P:(ct + 1) * P],
                                            idr[:])
                        if ct % 2 == 0:
                            nc.vector.tensor_copy(
                                xT[:, ct, tt * P:(tt + 1) * P],
                                pt[:].bitcast(dt.float32))
                        else:
                            nc.scalar.copy(xT[:, ct, tt * P:(tt + 1) * P],
                                           pt[:].bitcast(dt.float32))

                # qkvT[c, t] = Wv.T @ xT + bv  (per ct, two 512 chunks)
                for ct in range(NC_):
                    for ch in range(2):
                        sl = slice(ch * 512, (ch + 1) * 512)
                        pq = psa.tile([P, 512], dt.float32, tag="misc")
                        for k in range(NC_):
                            nc.tensor.matmul(
                                pq[:], wvt[:, k, ct * P:(ct + 1) * P],
                                xT[:, k, sl], start=(k == 0),
                                stop=(k == NC_ - 1))
                        nc.vector.tensor_copy(qkvT[:, ct, sl], pq[:])

                    # v2 blocks for this ct: transpose qkvT[ct] tiles
                    for tt in range(NT):
                        pt2 = psa.tile([P, P], f32r, tag="misc")
                        nc.tensor.transpose(
                            pt2[:], qkvT[:, ct, tt * P:(tt + 1) * P], idr[:])
                        dst = (v2[:, tt, :]
                               .rearrange("p (h c) -> p h c", c=HD + 1)
                               [:, 2 * ct:2 * ct + 2, 0:HD])
                        src = pt2[:].bitcast(dt.float32).rearrange(
                            "p (h c) -> p h c", c=HD)
                        nc.vector.tensor_scalar_mul(dst, src,
                                                    rw_col[:, tt:tt + 1])

            # ones columns of v2 (denominator trick)
            for tt in range(NT):
                v2h = v2[:, tt, :].rearrange("p (h c) -> p h c", c=HD + 1)
                nc.gpsimd.memset(v2h[:, :, HD:HD + 1], 1.0)

            # ---- phase 2: attention per head pair ----
            with tc.tile_pool(name="wp", bufs=1) as wpp:
                wpt = wpp.tile([P, NC_, C], f32r)     # [ci-part, k, co]
                for k in range(NC_):
                    nc.sync.dma_start(wpt[:, k, :],
                                      wp[k * P:(k + 1) * P, :].bitcast(f32r))

                def emit_av(g, pT_tiles):
                    for hh in range(2):
                        h = 2 * g + hh
                        for c in range(2):
                            py = psav.tile([P, 512], dt.float32, tag="av")
                            last = min(4 * c + 3, NT - 1)
                            for jt in range(last + 1):
                                a = max(c * 512, jt * P)
                                nc.tensor.matmul(
                                    py[0:HD + 1, a - c * 512:512],
                                    v2[:, jt, h * (HD + 1):
                                       (h + 1) * (HD + 1)],
                                    pT_tiles[jt][:, hh, a:(c + 1) * 512],
                                    start=(jt == 0), stop=(jt == last))
                            drow = smp.tile([1, 512], dt.float32, tag="dr")
                            nc.vector.tensor_copy(drow[:], py[HD:HD + 1, :])
                            rrow = smp.tile([1, 512], dt.float32, tag="rr")
                            nc.vector.reciprocal_approx_fast(
                                rrow[:], drow[:])
                            bc = smp.tile([HD, 512], dt.float32, tag="bc")
                            nc.gpsimd.partition_broadcast(bc[:], rrow[:])
                            nc.vector.tensor_mul(
                                yT[hh * HD:(hh + 1) * HD, g,
                                   c * 512:(c + 1) * 512],
                                py[0:HD, :], bc[:])

                prev = None
                for g in range(NPAIR):
                    pT_tiles = []
                    for jt in range(NT):
                        pt_t = pTp.tile([P, 2, T], dt.bfloat16, tag="pT")
                        pT_tiles.append(pt_t)
                        for (off, cw) in _chunks(jt):
                            pa = psatt.tile([P, 2, 512], dt.float32,
                                            tag="att")
                            for hh in range(2):
                                lo = hh * HD
                                nc.tensor.matmul(
                                    pa[:, hh, 0:cw],
                                    qkvT[lo:lo + HD, g,
                                         jt * P:(jt + 1) * P],
                                    qkvT[lo:lo + HD, g, off:off + cw],
                                    start=True, stop=True)
                            nc.scalar.activation(
                                pt_t[:, :, off:off + cw], pa[:, :, 0:cw],
                                AF.Exp, bias=0.0, scale=rwq[:, jt:jt + 1])
                        # mask the diagonal block (keep j <= i)
                        nc.vector.tensor_mul(
                            pt_t[:, :, jt * P:(jt + 1) * P],
                            pt_t[:, :, jt * P:(jt + 1) * P], mskv)

                    if prev is not None:
                        emit_av(*prev)
                    prev = (g, pT_tiles)

                if prev is not None:
                    emit_av(*prev)

                # ---- phase 3: proj ----
                for tt in range(NT):
                    for ch in range(2):
                        sl = slice(ch * 512, (ch + 1) * 512)
                        po = psa.tile([P, 512], dt.float32, tag="misc")
                        for k in range(NC_):
                            nc.tensor.matmul(
                                po[:], yT[:, k, tt * P:(tt + 1) * P],
                                wpt[:, k, sl], start=(k == 0),
                                stop=(k == NC_ - 1))
                        fo = fop.tile([P, 512], dt.float32, tag="fo")
                        nc.scalar.copy(fo[:], po[:])
                        nc.sync.dma_start(out[tt * P:(tt + 1) * P, sl], fo[:])

    nc.compile()
    return nc


_NC_CACHE = None


def _get_nc():
    global _NC_CACHE
    if _NC_CACHE is None:
        _NC_CACHE = build_nc()
    return _NC_CACHE


def _consts():
    tri = np.triu(np.ones((P, P), np.float32))  # keep j <= i
    return {
        "ident": np.eye(P, dtype=np.float32),
        "mask2": np.concatenate([tri, tri], axis=1)
                   .astype(ml_dtypes.bfloat16),
    }


def run(inputs, trace=False, tmpdir=None):
    x = np.asarray(inputs["x"], dtype=np.float32)
    rw = np.asarray(inputs["row_weights"], dtype=np.float32)
    wv = np.asarray(inputs["Wv"], dtype=np.float32)
    bv = np.asarray(inputs["bv"], dtype=np.float32)
    wp = np.asarray(inputs["Wproj"], dtype=np.float32)
    bp = np.asarray(inputs["bproj"], dtype=np.float32)
    assert not bv.any() and not bp.any(), \
        "kernel specialized for zero biases (spec: fill=zeros)"
    consts = _consts()
    in_maps = []
    for b in range(B):
        rwc = np.ascontiguousarray(rw[b].reshape(8, 128).T)
        m = {"x": np.ascontiguousarray(x[b]),
             "rw_col": rwc, "rwq": rwc * np.float32(SCALE),
             "wv": wv, "bv": bv, "wp": wp, "bp": bp}
        m.update(consts)
        in_maps.append(m)
    nc = _get_nc()
    res = run_bass_kernel_spmd(nc, in_maps, list(range(B)), trace=trace,
                               tmpdir=tmpdir)
    out = np.stack([res.results[b]["out"] for b in range(B)])
    return out, res


def kernel(**inputs):
    out, _ = run(inputs, trace=False)
    return out


if __name__ == "__main__":
    rng = np.random.default_rng(0)
    ins = {
        "x": rng.standard_normal((B, T, C)).astype(np.float32),
        "row_weights": rng.uniform(0.5, 1.5, (B, T)).astype(np.float32),
        "Wv": (rng.standard_normal((C, C)) * 0.02).astype(np.float32),
        "bv": np.zeros(C, np.float32),
        "Wproj": (rng.standard_normal((C, C)) * 0.02).astype(np.float32),
        "bproj": np.zeros(C, np.float32),
    }
    out = kernel(**ins)
    print("kernel ran, out", out.shape, out.dtype)
